# revision 14
# baseline (speedup 1.0000x reference)
"""Trainium2 Bass kernel: DifferentiableAddressingHead (NTM-style addressing).

V3: fp16 big-data path. Data parallel over batch (64 rows/core).

Per-core dataflow:
  m-mapping: m = 2048*ph + 16*p + 2*v + s  (ph: m-half, p: SBUF partition,
  v: 8 sub-rows, s: parity). Stage-C layout: rows r = 2b+s, cols
  c = 1024*ph + 128*v + p  <->  j = m//2 = 1024*ph + 8*p + v.
  Host interleaves prev_weights / de-interleaves the output.

  Big path (per b):
    one SWDGE cast-DMA mem[b] fp32 -> L [128, (ph v s d)] fp16
    (4KB contiguous runs per partition per ph).
    PE: 16 fp16 transposes [128,128] -> T psum fp16 [(s,d), p-cols]
    DVE: Ct = q-scaled copy of T (fp16, 2x from fp16 psum)
    DVE/ACT: St = Square(T) (fp16)
    PE: fp16 accumulate-MMs vs shared ones-master (cols 126/127 slide),
        rows 2b|2b+1 of dots/norms psum banks, start b==0 stop b==63.
  Stage C: cosine sim + softmax + gate + 3-tap circular conv (cross-parity
  via PE permutation matmuls SWP_E/SWP_O; j-shifts via strided v/p APs) +
  sharpen + normalize, in fp32, rows (2b+s).
"""

from contextlib import ExitStack

import numpy as np

import concourse.bass as bass
import concourse.tile as tile
from concourse import masks, mybir

B, M, D, C = 512, 4096, 64, 256
NCORES = 8
BL = B // NCORES          # 64 batch rows per core
NSHIFT = 3
EPS = 1e-8

F32 = mybir.dt.float32
F16 = mybir.dt.float16
AF = mybir.ActivationFunctionType
ALU = mybir.AluOpType
AX = mybir.AxisListType

P = 128
MH2 = M // 2              # 2048 stage-C columns
NPH = 2                   # m-half phases
NV = 8                    # v sub-rows per (ph, partition)

# fraction of St squares routed to DVE instead of ACT (engine balance):
# tile t goes to DVE when t % ST_MOD < ST_DVE
ST_MOD = 4
ST_DVE = 0   # TensorTensor cannot read 2 PSUM inputs; St stays on ACT


def _body(tc, nc, mem, cs, prev2, Wk, bb, wheads, out):
    ctx = tc._body_ctx

    const = ctx.enter_context(tc.tile_pool(name="const", bufs=1))
    small = ctx.enter_context(tc.tile_pool(name="small", bufs=1))
    spsum = ctx.enter_context(tc.tile_pool(name="spsum", bufs=1, space="PSUM"))
    accps = ctx.enter_context(tc.tile_pool(name="accps", bufs=1, space="PSUM"))
    tps = ctx.enter_context(tc.tile_pool(name="tps", bufs=3, space="PSUM"))
    mem_pool = ctx.enter_context(tc.tile_pool(name="mem", bufs=3))
    cs_pool = ctx.enter_context(tc.tile_pool(name="cspool", bufs=10))
    ss_pool = ctx.enter_context(tc.tile_pool(name="sspool", bufs=10))
    big = ctx.enter_context(tc.tile_pool(name="big", bufs=1))

    # ---------------- constants ----------------
    ident = const.tile([P, P], F32, tag="ident")
    masks.make_identity(nc, ident[:])
    ident16 = const.tile([P, P], F16, tag="ident16")
    nc.vector.tensor_copy(ident16[:], ident[:])
    ones_row = const.tile([1, P], F32, tag="ones")
    nc.gpsimd.memset(ones_row[:], 1.0)

    # ones-master for selector matmuls (fp16): col 126 = ones rows 0:64,
    # col 127 = ones rows 64:128; batch b uses slice [126-2b : 254-2b]
    master = const.tile([P, 2 * P], F16, tag="master")
    nc.vector.memset(master[:], 0.0)
    nc.vector.memset(master[0:64, 126:127], 1.0)
    nc.vector.memset(master[64:128, 127:128], 1.0)

    # D64 family: DE_f[b, 2b] = 1, DO_f[b, 2b+1] = 1, D64_f = DE + DO
    de_f = const.tile([BL, P], F32, tag="de_f")
    do_f = const.tile([BL, P], F32, tag="do_f")
    d64_f = const.tile([BL, P], F32, tag="d64_f")
    nc.vector.memset(de_f[:], 0.0)
    nc.vector.memset(do_f[:], 0.0)
    i64 = ident[0:BL, 0:BL]
    nc.vector.tensor_copy(
        de_f[:].rearrange("b (c two) -> b c two", two=2)[:, :, 0], i64)
    nc.vector.tensor_copy(
        do_f[:].rearrange("b (c two) -> b c two", two=2)[:, :, 1], i64)
    nc.vector.tensor_tensor(d64_f[:], de_f[:], do_f[:], op=ALU.add)
    # D64T [128, 64]: col b = ones at rows 2b, 2b+1 (for pair-sums)
    d64t_ps_t = spsum.tile([P, 64], F32, tag="sm", name="d64t_ps_t")
    d64t_ps = d64t_ps_t[:, 0:BL]
    nc.tensor.transpose(d64t_ps[:], d64_f[:], ident[0:BL, 0:BL])
    d64t = const.tile([P, BL], F32, tag="d64t")
    nc.vector.tensor_copy(d64t[:], d64t_ps[:])

    # parity-swap permutations (free-dim shifted identity halves):
    # gE = SWP_E^T @ g puts g[odd rows] onto even rows (zeros on odd), and
    # gO = SWP_O^T @ g puts g[even rows] onto odd rows.
    swpe = const.tile([P, P], F32, tag="swpe")
    swpo = const.tile([P, P], F32, tag="swpo")
    nc.vector.memset(swpe[:], 0.0)
    nc.vector.memset(swpo[:], 0.0)
    iv = ident[:].rearrange("p (c two) -> p c two", two=2)
    nc.vector.tensor_copy(
        swpe[:].rearrange("p (c two) -> p c two", two=2)[:, :, 0], iv[:, :, 1])
    nc.vector.tensor_copy(
        swpo[:].rearrange("p (c two) -> p c two", two=2)[:, :, 1], iv[:, :, 0])
    swpe16 = const.tile([P, P], F16, tag="swpe16")
    swpo16 = const.tile([P, P], F16, tag="swpo16")
    nc.vector.tensor_copy(swpe16[:], swpe[:])
    nc.vector.tensor_copy(swpo16[:], swpo[:])

    # ---------------- load controller + host-prepacked weights ----------
    cs_sb = small.tile([BL, C], F32, tag="cs")
    nc.sync.dma_start(cs_sb[:], cs[:])
    wk_sb = small.tile([P, 2 * D], F32, tag="wk")
    nc.sync.dma_start(wk_sb[:], Wk[:])           # wk_pack [128, 128]
    wh_sb = small.tile([P, 12], F32, tag="wh")
    nc.sync.dma_start(wh_sb[:], wheads[:])       # wh_pack [128, 12]
    brow = small.tile([1, 6], F32, tag="brow")
    nc.sync.dma_start(brow[:], bb[:])            # b_pack [1, 6]

    # ---------------- transpose cs -> csT [C(2x128 part), BL] --------------
    csT = small.tile([P, 2 * BL], F32, tag="csT")
    for ci in range(2):
        t_ps_t = spsum.tile([P, 64], F32, tag="sm", name="t_ps_t")
        t_ps = t_ps_t[:, 0:BL]
        nc.tensor.transpose(t_ps[:], cs_sb[:, ci * P:(ci + 1) * P], ident[0:BL, 0:BL])
        nc.vector.tensor_copy(csT[:, ci * BL:(ci + 1) * BL], t_ps[:])

    # ---------------- query + heads on PE ----------------
    q_ps_t = spsum.tile([P, 64], F32, tag="sm", name="q_ps_t")
    q_ps = q_ps_t[0:BL, 0:D]
    nc.tensor.matmul(q_ps[:], csT[:, 0:BL], wk_sb[:, 0:D], start=True, stop=False)
    nc.tensor.matmul(q_ps[:], csT[:, BL:2 * BL], wk_sb[:, D:2 * D],
                     start=False, stop=True)
    q_sb = small.tile([BL, D], F32, tag="qsb")
    nc.vector.tensor_copy(q_sb[:], q_ps[:])

    h_ps_t = spsum.tile([P, 64], F32, tag="sm", name="h_ps_t")
    h_ps = h_ps_t[0:BL, 0:6]
    nc.tensor.matmul(h_ps[:], csT[:, 0:BL], wh_sb[:, 0:6], start=True, stop=False)
    nc.tensor.matmul(h_ps[:], csT[:, BL:2 * BL], wh_sb[:, 6:12],
                     start=False, stop=False)
    nc.tensor.matmul(h_ps[:], ones_row[0:1, 0:BL], brow[:], start=False, stop=True)
    h_sb = small.tile([BL, 6], F32, tag="hsb")
    nc.vector.tensor_copy(h_sb[:], h_ps[:])

    # ---------------- per-batch scalars ([BL, 1]) ----------------
    qsq = small.tile([BL, D], F32, tag="qsq")
    qn2 = small.tile([BL, 1], F32, tag="qn2")
    nc.scalar.activation(qsq[:], q_sb[:], AF.Square, accum_out=qn2[:])
    qnorm = small.tile([BL, 1], F32, tag="qnorm")
    nc.scalar.activation(qnorm[:], qn2[:], AF.Sqrt)
    qne = small.tile([BL, 1], F32, tag="qne")
    nc.vector.tensor_scalar(qne[:], qnorm[:], EPS, None, op0=ALU.add)
    qrecip = small.tile([BL, 1], F32, tag="qrecip")
    nc.vector.reciprocal(qrecip[:], qne[:])

    # bscale = (softplus(h0)+1) / (|q|+eps)
    spe = small.tile([BL, 1], F32, tag="spe")
    nc.scalar.activation(spe[:], h_sb[:, 0:1], AF.Exp)
    spb = small.tile([BL, 1], F32, tag="spb")
    nc.scalar.activation(spb[:], spe[:], AF.Ln, bias=1.0)
    bscale = small.tile([BL, 1], F32, tag="bscale")
    nc.vector.tensor_scalar(bscale[:], spb[:], 1.0, qrecip[:],
                            op0=ALU.add, op1=ALU.mult)

    g_t = small.tile([BL, 1], F32, tag="gate")
    nc.scalar.activation(g_t[:], h_sb[:, 1:2], AF.Sigmoid)
    omg = small.tile([BL, 1], F32, tag="omg")
    nc.scalar.activation(omg[:], g_t[:], AF.Copy, bias=1.0, scale=-1.0)

    e3 = small.tile([BL, NSHIFT], F32, tag="e3")
    nc.scalar.activation(e3[:], h_sb[:, 2:5], AF.Exp)
    ssum = small.tile([BL, 1], F32, tag="ssum")
    nc.vector.tensor_reduce(ssum[:], e3[:], axis=AX.X, op=ALU.add)
    srec = small.tile([BL, 1], F32, tag="srec")
    nc.vector.reciprocal(srec[:], ssum[:])
    sk = small.tile([BL, NSHIFT], F32, tag="sk")
    nc.vector.tensor_scalar(sk[:], e3[:], srec[:], None, op0=ALU.mult)

    gse = small.tile([BL, 1], F32, tag="gse")
    nc.scalar.activation(gse[:], h_sb[:, 5:6], AF.Exp)
    gsp = small.tile([BL, 1], F32, tag="gsp")
    nc.scalar.activation(gsp[:], gse[:], AF.Ln, bias=1.0)
    gamma = small.tile([BL, 1], F32, tag="gamma")
    nc.vector.tensor_scalar(gamma[:], gsp[:], 1.0, None, op0=ALU.add)

    # ---------------- duplicate scalars to (2b+s) rows via PE ------------
    # scal_pack cols: bscale, gate, omg, gamma, sk0, sk1, sk2
    scal_pack = small.tile([BL, 7], F32, tag="scal_pack")
    nc.vector.tensor_copy(scal_pack[:, 0:1], bscale[:])
    nc.vector.tensor_copy(scal_pack[:, 1:2], g_t[:])
    nc.vector.tensor_copy(scal_pack[:, 2:3], omg[:])
    nc.vector.tensor_copy(scal_pack[:, 3:4], gamma[:])
    nc.vector.tensor_copy(scal_pack[:, 4:7], sk[:])
    scal2_ps_t = spsum.tile([P, 64], F32, tag="sm", name="scal2_ps_t")
    scal2_ps = scal2_ps_t[:, 0:7]
    nc.tensor.matmul(scal2_ps[:], d64_f[:], scal_pack[:], start=True, stop=True)
    scal2 = small.tile([P, 7], F32, tag="scal2")
    nc.vector.tensor_copy(scal2[:], scal2_ps[:])
    bscale2 = scal2[:, 0:1]
    g2 = scal2[:, 1:2]
    omg2 = scal2[:, 2:3]
    gamma2 = scal2[:, 3:4]
    sk1_2 = scal2[:, 6 - 1:6]          # sk1 at col 5

    # conv cross-term coefficients:
    # skU[2b] = sk2_b, skU[2b+1] = sk0_b (unshifted cross term)
    # skS[2b] = sk0_b (shift -1 on gE), skS[2b+1] = sk2_b (shift +1 on gO)
    skx = small.tile([BL, 2], F32, tag="skx")
    nc.vector.tensor_copy(skx[:, 0:1], sk[:, 2:3])
    nc.vector.tensor_copy(skx[:, 1:2], sk[:, 0:1])
    skx2 = small.tile([BL, 2], F32, tag="skx2")
    nc.vector.tensor_copy(skx2[:, 0:1], sk[:, 0:1])
    nc.vector.tensor_copy(skx2[:, 1:2], sk[:, 2:3])
    sk2ps_t = spsum.tile([P, 64], F32, tag="sm", name="sk2ps_t")
    sk2ps = sk2ps_t[:, 0:2]
    nc.tensor.matmul(sk2ps[:], de_f[:], skx[:], start=True, stop=False)
    nc.tensor.matmul(sk2ps[:], do_f[:], skx2[:], start=False, stop=True)
    skuv = small.tile([P, 2], F32, tag="skuv")
    nc.vector.tensor_copy(skuv[:], sk2ps[:])
    skU = skuv[:, 0:1]
    skS = skuv[:, 1:2]

    # q2 [128, 64] fp32: col b = [q_b ; q_b] (drain scale operand)
    qT_ps_t = spsum.tile([P, 64], F32, tag="sm", name="qT_ps_t")
    qT_ps = qT_ps_t[0:BL, 0:BL]
    nc.tensor.transpose(qT_ps[:], q_sb[:], ident[0:BL, 0:BL])
    q2 = small.tile([P, BL], F32, tag="q2")
    nc.vector.tensor_copy(q2[0:64, :], qT_ps[:])
    nc.vector.tensor_copy(q2[64:128, :], qT_ps[:])

    # ---------------- stage B ----------------
    d_dot = big.tile([P, MH2], F32, tag="ddot")
    d_nsq = big.tile([P, MH2], F32, tag="dnsq")
    dd16 = big.tile([P, MH2], F16, tag="dd16")     # exp'd numerators (fp16)
    prev16 = big.tile([P, MH2], F16, tag="prev16")
    cp16 = big.tile([P, MH2], F16, tag="cp16")     # conv(prev)
    cd16 = big.tile([P, MH2], F16, tag="cd16")     # conv(exp)
    gEp = big.tile([P, MH2], F16, tag="gEp")
    gOp = big.tile([P, MH2], F16, tag="gOp")
    gEd = big.tile([P, MH2], F16, tag="gEd")
    gOd = big.tile([P, MH2], F16, tag="gOd")
    nc.gpsimd.dma_start(prev16[:], prev2[:])       # cast fp32->fp16

    def vview(t, ph, vlo, vhi, plo, phi):
        # view t[:, 1024*ph + 128*v + p] for v in [vlo,vhi), p in [plo,phi)
        return t[:, 1024 * ph:1024 * (ph + 1)].rearrange(
            "r (v p) -> r v p", v=NV)[:, vlo:vhi, plo:phi]

    def emit_conv_half(src, gEt, gOt, dst, ph, nm):
        """dst[hsl] = 3-tap circular conv of src for the ph half, except the
        4 single-column cross-half/circular terms (emit_conv_fixups)."""
        hsl = slice(1024 * ph, 1024 * (ph + 1))
        for ci, c0 in enumerate(range(1024 * ph, 1024 * (ph + 1), 512)):
            sp = tps.tile([P, 512], F32, tag="T", name=f"swpE_{nm}{ph}{ci}")
            nc.tensor.matmul(sp[:], swpe16[:], src[:, c0:c0 + 512],
                             start=True, stop=True)
            nc.vector.tensor_copy(gEt[:, c0:c0 + 512], sp[:])
            sp2 = tps.tile([P, 512], F32, tag="T", name=f"swpO_{nm}{ph}{ci}")
            nc.tensor.matmul(sp2[:], swpo16[:], src[:, c0:c0 + 512],
                             start=True, stop=True)
            nc.vector.tensor_copy(gOt[:, c0:c0 + 512], sp2[:])
        nc.vector.tensor_scalar(dst[:, hsl], src[:, hsl], sk1_2, None,
                                op0=ALU.mult)
        nc.vector.scalar_tensor_tensor(dst[:, hsl], gEt[:, hsl], skU,
                                       dst[:, hsl], op0=ALU.mult, op1=ALU.add)
        nc.vector.scalar_tensor_tensor(dst[:, hsl], gOt[:, hsl], skU,
                                       dst[:, hsl], op0=ALU.mult, op1=ALU.add)
        # S+ shifted gE (src col j-1 -> dst col j)
        nc.vector.scalar_tensor_tensor(
            vview(dst, ph, 1, NV, 0, P), vview(gEt, ph, 0, NV - 1, 0, P), skS,
            vview(dst, ph, 1, NV, 0, P), op0=ALU.mult, op1=ALU.add)
        nc.vector.scalar_tensor_tensor(
            vview(dst, ph, 0, 1, 1, P), vview(gEt, ph, NV - 1, NV, 0, P - 1),
            skS, vview(dst, ph, 0, 1, 1, P), op0=ALU.mult, op1=ALU.add)
        # S- shifted gO (src col j+1 -> dst col j)
        nc.vector.scalar_tensor_tensor(
            vview(dst, ph, 0, NV - 1, 0, P), vview(gOt, ph, 1, NV, 0, P), skS,
            vview(dst, ph, 0, NV - 1, 0, P), op0=ALU.mult, op1=ALU.add)
        nc.vector.scalar_tensor_tensor(
            vview(dst, ph, NV - 1, NV, 0, P - 1), vview(gOt, ph, 0, 1, 1, P),
            skS, vview(dst, ph, NV - 1, NV, 0, P - 1),
            op0=ALU.mult, op1=ALU.add)

    def emit_conv_fixups(gEt, gOt, dst):
        # ph boundary: dst j=1024 <- src j=1023 ; circular: dst j=0 <- j=2047
        nc.vector.scalar_tensor_tensor(dst[:, 1024:1025], gEt[:, 1023:1024],
                                       skS, dst[:, 1024:1025],
                                       op0=ALU.mult, op1=ALU.add)
        nc.vector.scalar_tensor_tensor(dst[:, 0:1], gEt[:, MH2 - 1:MH2], skS,
                                       dst[:, 0:1], op0=ALU.mult, op1=ALU.add)
        # mirror for gO: dst j=1023 <- j=1024 ; dst j=2047 <- j=0
        nc.vector.scalar_tensor_tensor(dst[:, 1023:1024], gOt[:, 1024:1025],
                                       skS, dst[:, 1023:1024],
                                       op0=ALU.mult, op1=ALU.add)
        nc.vector.scalar_tensor_tensor(dst[:, MH2 - 1:MH2], gOt[:, 0:1], skS,
                                       dst[:, MH2 - 1:MH2],
                                       op0=ALU.mult, op1=ALU.add)

    LAG = 6  # k-tiles the accumulation matmuls trail the transposes by
    tile_idx = 0
    # L tiles are per-(ph,b) [128, (v s d)] fp16; partition line = one
    # contiguous 4KB fp32 run in HBM (m-rows 16p..16p+16 of the ph half).
    lv_all = mem[:].rearrange("b (ph p v s) d -> b ph p (v s d)",
                              ph=NPH, p=P, v=NV, s=2)

    for ph in range(NPH):
        dbank = [accps.tile([P, 512], F32, tag=f"dot{k}", name=f"dot{k}") for k in range(2)]
        nbank = [accps.tile([P, 512], F32, tag=f"nrm{k}", name=f"nrm{k}") for k in range(2)]

        def emit_mm(item):
            Ct, St, b_, k_ = item
            msl = master[:, 126 - 2 * b_:254 - 2 * b_]
            nc.tensor.matmul(dbank[k_][:], msl, Ct[:],
                             start=(b_ == 0), stop=(b_ == BL - 1),
                             skip_group_check=True)
            nc.tensor.matmul(nbank[k_][:], msl, St[:],
                             start=(b_ == 0), stop=(b_ == BL - 1),
                             skip_group_check=True)

        pend = []
        GRP = 8   # batches per cast-DMA (amortizes SWDGE overhead)
        for b in range(BL):
            if b % GRP == 0:
                Lg = mem_pool.tile([P, GRP * 1024], F16, tag="L")
                nc.gpsimd.dma_start(
                    Lg[:].rearrange("p (g f) -> p g f", g=GRP),
                    lv_all[b:b + GRP, ph].rearrange("g p f -> p g f"))
            L = Lg[:, (b % GRP) * 1024:(b % GRP + 1) * 1024]

            for k in range(2):
                T = tps.tile([P, 512], F16, tag="T")
                for tt in range(4):
                    v = 4 * k + tt
                    nc.tensor.transpose(T[:, tt * P:(tt + 1) * P],
                                        L[:, v * P:(v + 1) * P], ident16[:])
                Ct = cs_pool.tile([P, 512], F16, tag="C")
                St = ss_pool.tile([P, 512], F16, tag="S")
                nc.vector.tensor_scalar(Ct[:], T[:], q2[:, b:b + 1], None,
                                        op0=ALU.mult)
                if tile_idx % ST_MOD < ST_DVE:
                    nc.vector.tensor_tensor(St[:], T[:], T[:], op=ALU.mult)
                else:
                    nc.scalar.activation(St[:], T[:], AF.Square)
                pend.append((Ct, St, b, k))
                if len(pend) > LAG:
                    emit_mm(pend.pop(0))
                tile_idx += 1
        for item in pend:
            emit_mm(item)
        pend = []

        for k in range(2):
            c0 = 1024 * ph + 512 * k
            nc.vector.tensor_copy(d_dot[:, c0:c0 + 512], dbank[k][:])
            nc.scalar.activation(d_nsq[:, c0:c0 + 512], nbank[k][:], AF.Copy)

        # ---------------- stage C on this m-half as soon as it drains ----
        h0 = 1024 * ph
        dd = d_dot[:, h0:h0 + 1024]
        dn = d_nsq[:, h0:h0 + 1024]
        # sim = dot * rsqrt(nsq) * bscale ; softmax numerator exp(beta*cos)
        nc.scalar.activation(dn[:], dn[:], AF.Ln)
        nc.scalar.activation(dn[:], dn[:], AF.Exp, scale=-0.5)
        nc.vector.tensor_tensor(dd[:], dd[:], dn[:], op=ALU.mult)
        nc.scalar.activation(dd16[:, h0:h0 + 1024], dd[:], AF.Exp,
                             scale=bscale2)
        # conv(exp) for this half; conv(prev) emitted once to overlap ph=1
        emit_conv_half(dd16, gEd, gOd, cd16, ph, "d")
        if ph == 0:
            for php in range(NPH):
                emit_conv_half(prev16, gEp, gOp, cp16, php, "p")
            emit_conv_fixups(gEp, gOp, cp16)

    # ---------------- stage C tail: softmax norm, gate, conv, sharpen ----
    emit_conv_fixups(gEd, gOd, cd16)
    esum_h = small.tile([P, 2], F32, tag="esum_h")
    for ph in range(NPH):
        nc.vector.tensor_reduce(esum_h[:, ph:ph + 1],
                                dd16[:, 1024 * ph:1024 * (ph + 1)],
                                axis=AX.X, op=ALU.add)
    esum = small.tile([P, 1], F32, tag="esum")
    nc.vector.tensor_reduce(esum[:], esum_h[:], axis=AX.X, op=ALU.add)
    eps_ps_t = spsum.tile([P, 64], F32, tag="sm", name="eps_ps_t")
    eps_ps = eps_ps_t[0:BL, 0:1]
    nc.tensor.matmul(eps_ps[:], d64t[:], esum[:], start=True, stop=True)
    erec64 = small.tile([BL, 1], F32, tag="erec64")
    nc.vector.reciprocal(erec64[:], eps_ps[:])
    er_ps_t = spsum.tile([P, 64], F32, tag="sm", name="er_ps_t")
    er_ps = er_ps_t[:, 0:1]
    nc.tensor.matmul(er_ps[:], d64_f[:], erec64[:], start=True, stop=True)
    erec2 = small.tile([P, 1], F32, tag="erec2")
    nc.vector.tensor_copy(erec2[:], er_ps[:])
    galpha2 = small.tile([P, 1], F32, tag="galpha2")
    nc.vector.tensor_tensor(galpha2[:], g2, erec2[:], op=ALU.mult)

    # combine: conv(gated) = galpha*conv(exp) + omg*conv(prev), then
    # sharpen (cv+eps)^gamma and normalize.  sh reuses d_nsq (fp32 scratch).
    t16 = big.tile([P, MH2], F16, tag="t16")
    nc.vector.tensor_scalar(t16[:], cd16[:], galpha2[:, 0:1], None,
                            op0=ALU.mult)
    nc.vector.scalar_tensor_tensor(cp16[:], cp16[:], omg2, t16[:],
                                   op0=ALU.mult, op1=ALU.add)
    sh = d_nsq
    nc.scalar.activation(sh[:], cp16[:], AF.Ln, bias=EPS)
    nc.scalar.activation(sh[:], sh[:], AF.Exp, scale=gamma2)
    psumt = small.tile([P, 1], F32, tag="psumt")
    nc.vector.tensor_reduce(psumt[:], sh[:], axis=AX.X, op=ALU.add)
    pp_ps_t = spsum.tile([P, 64], F32, tag="sm", name="pp_ps_t")
    pp_ps = pp_ps_t[0:BL, 0:1]
    nc.tensor.matmul(pp_ps[:], d64t[:], psumt[:], start=True, stop=True)
    ppe = small.tile([BL, 1], F32, tag="ppe")
    nc.vector.tensor_scalar(ppe[:], pp_ps[:], EPS, None, op0=ALU.add)
    prc64 = small.tile([BL, 1], F32, tag="prc64")
    nc.vector.reciprocal(prc64[:], ppe[:])
    pr_ps_t = spsum.tile([P, 64], F32, tag="sm", name="pr_ps_t")
    pr_ps = pr_ps_t[:, 0:1]
    nc.tensor.matmul(pr_ps[:], d64_f[:], prc64[:], start=True, stop=True)
    prc2 = small.tile([P, 1], F32, tag="prc2")
    nc.vector.tensor_copy(prc2[:], pr_ps[:])

    nc.scalar.activation(sh[:], sh[:], AF.Copy, scale=prc2[:])
    nc.sync.dma_start(out[:], sh[:])


def build(split_waits=True):
    nc = bass.Bass()
    mem = nc.dram_tensor("memory", [BL, M, D], F32, kind="ExternalInput")
    cs = nc.dram_tensor("controller_state", [BL, C], F32, kind="ExternalInput")
    prev2 = nc.dram_tensor("prev2", [P, MH2], F32, kind="ExternalInput")
    wk_pack = nc.dram_tensor("wk_pack", [P, 2 * D], F32, kind="ExternalInput")
    wh_pack = nc.dram_tensor("wh_pack", [P, 12], F32, kind="ExternalInput")
    b_pack = nc.dram_tensor("b_pack", [1, 6], F32, kind="ExternalInput")
    out = nc.dram_tensor("out", [P, MH2], F32, kind="ExternalOutput")

    eps_t = nc.alloc_sbuf_tensor("const-f32-eps", [128, 1], F32)
    nc.gpsimd.memset(eps_t.ap(), EPS)
    nc.const_aps.aps[(F32, EPS)] = eps_t.ap()
    nc.all_engine_barrier()

    with tile.TileContext(nc) as tc:
        with ExitStack() as ctx:
            tc._body_ctx = ctx
            _body(tc, nc, mem, cs, prev2, wk_pack, b_pack, wh_pack, out)
    if split_waits:
        _split_multiwait(nc)
    return nc


def _split_multiwait(nc, max_waits=1):
    """Walrus ISA structs encode a limited number of semaphore waits per
    instruction. Move all but one wait of any multi-wait instruction onto
    same-engine InstNoOp instructions inserted directly before it."""
    for fn in nc.m.functions:
        for blk in fn.blocks:
            insts = blk.instructions
            idx = 0
            while idx < len(insts):
                inst = insts[idx]
                si = inst.sync_info
                if si is not None and len(si.on_wait) > max_waits:
                    waits = list(si.on_wait)
                    extra, keep = waits[:-max_waits], waits[-max_waits:]
                    for w in extra:
                        nop = mybir.InstNoOp(
                            name=nc.get_next_instruction_name(),
                            sync_info=mybir.SyncInfo(on_wait=[w], on_update=[]),
                            bass_nofuse=True,
                            engine=inst.engine,
                        )
                        insts.insert(idx, nop)
                        idx += 1
                    inst.sync_info = mybir.SyncInfo(
                        on_wait=keep, on_update=list(si.on_update))
                idx += 1


_NC = None


def _get_nc():
    global _NC
    if _NC is None:
        _NC = build()
    return _NC


def _j_of_c():
    # j = 1024*ph + 8*p + v for c = 1024*ph + 128*v + p
    c = np.arange(MH2)
    ph = c // 1024
    v = (c % 1024) // P
    p = c % P
    return 1024 * ph + 8 * p + v


_JC = _j_of_c()


def _make_in_maps(inputs):
    full = {k: np.ascontiguousarray(np.asarray(v, dtype=np.float32))
            for k, v in inputs.items()}
    wk_pack = np.ascontiguousarray(
        np.concatenate([full["Wk"][0:P, :], full["Wk"][P:C, :]], axis=1))
    wh = np.concatenate(
        [full["Wb"], full["Wgate"], full["Ws"], full["Wg"]], axis=1)
    wh_pack = np.ascontiguousarray(np.concatenate([wh[0:P], wh[P:C]], axis=1))
    b_pack = np.ascontiguousarray(np.concatenate(
        [full["bb"].reshape(-1), full["bgate"].reshape(-1),
         full["bs"].reshape(-1), full["bg"].reshape(-1)]).reshape(1, 6))
    in_maps = []
    for c in range(NCORES):
        sl = slice(c * BL, (c + 1) * BL)
        # prev2 rows 2b+s = prev[b, 2*j(c)+s]
        pv = full["previous_weights"][sl]            # [64, 4096]
        pj = pv.reshape(BL, MH2, 2)[:, _JC, :]       # [64, 2048(c), 2(s)]
        prev2 = np.ascontiguousarray(
            pj.transpose(0, 2, 1).reshape(P, MH2))
        in_maps.append({
            "memory": full["memory"][sl],
            "controller_state": full["controller_state"][sl],
            "prev2": prev2,
            "wk_pack": wk_pack, "wh_pack": wh_pack, "b_pack": b_pack,
        })
    return in_maps


def run(inputs, **kwargs):
    from concourse.bass_utils import run_bass_kernel_spmd
    nc = _get_nc()
    res = run_bass_kernel_spmd(nc, _make_in_maps(inputs),
                               list(range(NCORES)), **kwargs)
    inv = np.argsort(_JC)
    outs = []
    for c in range(NCORES):
        o2 = res.results[c]["out"]                   # [128, 2048] rows 2b+s
        o = o2.reshape(BL, 2, MH2)[:, :, inv]        # cols back to j-order
        o = o.transpose(0, 2, 1).reshape(BL, M)      # m = 2j+s
        outs.append(o)
    out = np.concatenate(outs, axis=0)
    return out.astype(np.float32), res


def kernel(**inputs):
    out, _ = run(inputs)
    return out


# revision 18
# speedup vs baseline: 1.0323x; 1.0323x over previous
"""Trainium2 Bass kernel: DifferentiableAddressingHead (NTM-style addressing).

V3: fp16 big-data path. Data parallel over batch (64 rows/core).

Per-core dataflow:
  m-mapping: m = 2048*ph + 16*p + 2*v + s  (ph: m-half, p: SBUF partition,
  v: 8 sub-rows, s: parity). Stage-C layout: rows r = 2b+s, cols
  c = 1024*ph + 128*v + p  <->  j = m//2 = 1024*ph + 8*p + v.
  Host interleaves prev_weights / de-interleaves the output.

  Big path (per b):
    one SWDGE cast-DMA mem[b] fp32 -> L [128, (ph v s d)] fp16
    (4KB contiguous runs per partition per ph).
    PE: 16 fp16 transposes [128,128] -> T psum fp16 [(s,d), p-cols]
    DVE: Ct = q-scaled copy of T (fp16, 2x from fp16 psum)
    DVE/ACT: St = Square(T) (fp16)
    PE: fp16 accumulate-MMs vs shared ones-master (cols 126/127 slide),
        rows 2b|2b+1 of dots/norms psum banks, start b==0 stop b==63.
  Stage C: cosine sim + softmax + gate + 3-tap circular conv (cross-parity
  via PE permutation matmuls SWP_E/SWP_O; j-shifts via strided v/p APs) +
  sharpen + normalize, in fp32, rows (2b+s).
"""

from contextlib import ExitStack

import numpy as np

import concourse.bass as bass
import concourse.tile as tile
from concourse import masks, mybir

B, M, D, C = 512, 4096, 64, 256
NCORES = 8
BL = B // NCORES          # 64 batch rows per core
NSHIFT = 3
EPS = 1e-8

F32 = mybir.dt.float32
F16 = mybir.dt.float16
AF = mybir.ActivationFunctionType
ALU = mybir.AluOpType
AX = mybir.AxisListType

P = 128
MH2 = M // 2              # 2048 stage-C columns
NPH = 2                   # m-half phases
NV = 8                    # v sub-rows per (ph, partition)

# fraction of St squares routed to DVE instead of ACT (engine balance):
# tile t goes to DVE when t % ST_MOD < ST_DVE
ST_MOD = 4
ST_DVE = 0   # TensorTensor cannot read 2 PSUM inputs; St stays on ACT


def _body(tc, nc, mem, cs, prev2, Wk, bb, wheads, out):
    ctx = tc._body_ctx

    const = ctx.enter_context(tc.tile_pool(name="const", bufs=1))
    small = ctx.enter_context(tc.tile_pool(name="small", bufs=1))
    spsum = ctx.enter_context(tc.tile_pool(name="spsum", bufs=1, space="PSUM"))
    accps = ctx.enter_context(tc.tile_pool(name="accps", bufs=1, space="PSUM"))
    tps = ctx.enter_context(tc.tile_pool(name="tps", bufs=3, space="PSUM"))
    mem_pool = ctx.enter_context(tc.tile_pool(name="mem", bufs=3))
    cs_pool = ctx.enter_context(tc.tile_pool(name="cspool", bufs=10))
    ss_pool = ctx.enter_context(tc.tile_pool(name="sspool", bufs=10))
    big = ctx.enter_context(tc.tile_pool(name="big", bufs=1))

    # ---------------- constants ----------------
    ident = const.tile([P, P], F32, tag="ident")
    masks.make_identity(nc, ident[:])
    ident16 = const.tile([P, P], F16, tag="ident16")
    nc.vector.tensor_copy(ident16[:], ident[:])
    ones_row = const.tile([1, P], F32, tag="ones")
    nc.gpsimd.memset(ones_row[:], 1.0)

    # ones-master for selector matmuls (fp16): col 126 = ones rows 0:64,
    # col 127 = ones rows 64:128; batch b uses slice [126-2b : 254-2b]
    master = const.tile([P, 2 * P], F16, tag="master")
    nc.vector.memset(master[:], 0.0)
    nc.vector.memset(master[0:64, 126:127], 1.0)
    nc.vector.memset(master[64:128, 127:128], 1.0)

    # D64 family: DE_f[b, 2b] = 1, DO_f[b, 2b+1] = 1, D64_f = DE + DO
    de_f = const.tile([BL, P], F32, tag="de_f")
    do_f = const.tile([BL, P], F32, tag="do_f")
    d64_f = const.tile([BL, P], F32, tag="d64_f")
    nc.vector.memset(de_f[:], 0.0)
    nc.vector.memset(do_f[:], 0.0)
    i64 = ident[0:BL, 0:BL]
    nc.vector.tensor_copy(
        de_f[:].rearrange("b (c two) -> b c two", two=2)[:, :, 0], i64)
    nc.vector.tensor_copy(
        do_f[:].rearrange("b (c two) -> b c two", two=2)[:, :, 1], i64)
    nc.vector.tensor_tensor(d64_f[:], de_f[:], do_f[:], op=ALU.add)
    # D64T [128, 64]: col b = ones at rows 2b, 2b+1 (for pair-sums)
    d64t_ps_t = spsum.tile([P, 64], F32, tag="sm", name="d64t_ps_t")
    d64t_ps = d64t_ps_t[:, 0:BL]
    nc.tensor.transpose(d64t_ps[:], d64_f[:], ident[0:BL, 0:BL])
    d64t = const.tile([P, BL], F32, tag="d64t")
    nc.vector.tensor_copy(d64t[:], d64t_ps[:])

    # parity-swap permutations (free-dim shifted identity halves):
    # gE = SWP_E^T @ g puts g[odd rows] onto even rows (zeros on odd), and
    # gO = SWP_O^T @ g puts g[even rows] onto odd rows.
    swpe = const.tile([P, P], F32, tag="swpe")
    swpo = const.tile([P, P], F32, tag="swpo")
    nc.vector.memset(swpe[:], 0.0)
    nc.vector.memset(swpo[:], 0.0)
    iv = ident[:].rearrange("p (c two) -> p c two", two=2)
    nc.vector.tensor_copy(
        swpe[:].rearrange("p (c two) -> p c two", two=2)[:, :, 0], iv[:, :, 1])
    nc.vector.tensor_copy(
        swpo[:].rearrange("p (c two) -> p c two", two=2)[:, :, 1], iv[:, :, 0])
    swpe16 = const.tile([P, P], F16, tag="swpe16")
    swpo16 = const.tile([P, P], F16, tag="swpo16")
    nc.vector.tensor_copy(swpe16[:], swpe[:])
    nc.vector.tensor_copy(swpo16[:], swpo[:])

    # ---------------- load controller + host-prepacked weights ----------
    cs_sb = small.tile([BL, C], F32, tag="cs")
    nc.sync.dma_start(cs_sb[:], cs[:])
    wk_sb = small.tile([P, 2 * D], F32, tag="wk")
    nc.sync.dma_start(wk_sb[:], Wk[:])           # wk_pack [128, 128]
    wh_sb = small.tile([P, 12], F32, tag="wh")
    nc.sync.dma_start(wh_sb[:], wheads[:])       # wh_pack [128, 12]
    brow = small.tile([1, 6], F32, tag="brow")
    nc.sync.dma_start(brow[:], bb[:])            # b_pack [1, 6]

    # ---------------- transpose cs -> csT [C(2x128 part), BL] --------------
    csT = small.tile([P, 2 * BL], F32, tag="csT")
    for ci in range(2):
        t_ps_t = spsum.tile([P, 64], F32, tag="sm", name="t_ps_t")
        t_ps = t_ps_t[:, 0:BL]
        nc.tensor.transpose(t_ps[:], cs_sb[:, ci * P:(ci + 1) * P], ident[0:BL, 0:BL])
        nc.vector.tensor_copy(csT[:, ci * BL:(ci + 1) * BL], t_ps[:])

    # ---------------- query + heads on PE ----------------
    q_ps_t = spsum.tile([P, 64], F32, tag="sm", name="q_ps_t")
    q_ps = q_ps_t[0:BL, 0:D]
    nc.tensor.matmul(q_ps[:], csT[:, 0:BL], wk_sb[:, 0:D], start=True, stop=False)
    nc.tensor.matmul(q_ps[:], csT[:, BL:2 * BL], wk_sb[:, D:2 * D],
                     start=False, stop=True)
    q_sb = small.tile([BL, D], F32, tag="qsb")
    nc.vector.tensor_copy(q_sb[:], q_ps[:])

    h_ps_t = spsum.tile([P, 64], F32, tag="sm", name="h_ps_t")
    h_ps = h_ps_t[0:BL, 0:6]
    nc.tensor.matmul(h_ps[:], csT[:, 0:BL], wh_sb[:, 0:6], start=True, stop=False)
    nc.tensor.matmul(h_ps[:], csT[:, BL:2 * BL], wh_sb[:, 6:12],
                     start=False, stop=False)
    nc.tensor.matmul(h_ps[:], ones_row[0:1, 0:BL], brow[:], start=False, stop=True)
    h_sb = small.tile([BL, 6], F32, tag="hsb")
    nc.vector.tensor_copy(h_sb[:], h_ps[:])

    # ---------------- per-batch scalars ([BL, 1]) ----------------
    qsq = small.tile([BL, D], F32, tag="qsq")
    qn2 = small.tile([BL, 1], F32, tag="qn2")
    nc.scalar.activation(qsq[:], q_sb[:], AF.Square, accum_out=qn2[:])
    qnorm = small.tile([BL, 1], F32, tag="qnorm")
    nc.scalar.activation(qnorm[:], qn2[:], AF.Sqrt)
    qne = small.tile([BL, 1], F32, tag="qne")
    nc.vector.tensor_scalar(qne[:], qnorm[:], EPS, None, op0=ALU.add)
    qrecip = small.tile([BL, 1], F32, tag="qrecip")
    nc.vector.reciprocal(qrecip[:], qne[:])

    # bscale = (softplus(h0)+1) / (|q|+eps)
    spe = small.tile([BL, 1], F32, tag="spe")
    nc.scalar.activation(spe[:], h_sb[:, 0:1], AF.Exp)
    spb = small.tile([BL, 1], F32, tag="spb")
    nc.scalar.activation(spb[:], spe[:], AF.Ln, bias=1.0)
    bscale = small.tile([BL, 1], F32, tag="bscale")
    nc.vector.tensor_scalar(bscale[:], spb[:], 1.0, qrecip[:],
                            op0=ALU.add, op1=ALU.mult)

    g_t = small.tile([BL, 1], F32, tag="gate")
    nc.scalar.activation(g_t[:], h_sb[:, 1:2], AF.Sigmoid)
    omg = small.tile([BL, 1], F32, tag="omg")
    nc.scalar.activation(omg[:], g_t[:], AF.Copy, bias=1.0, scale=-1.0)

    e3 = small.tile([BL, NSHIFT], F32, tag="e3")
    nc.scalar.activation(e3[:], h_sb[:, 2:5], AF.Exp)
    ssum = small.tile([BL, 1], F32, tag="ssum")
    nc.vector.tensor_reduce(ssum[:], e3[:], axis=AX.X, op=ALU.add)
    srec = small.tile([BL, 1], F32, tag="srec")
    nc.vector.reciprocal(srec[:], ssum[:])
    sk = small.tile([BL, NSHIFT], F32, tag="sk")
    nc.vector.tensor_scalar(sk[:], e3[:], srec[:], None, op0=ALU.mult)

    gse = small.tile([BL, 1], F32, tag="gse")
    nc.scalar.activation(gse[:], h_sb[:, 5:6], AF.Exp)
    gsp = small.tile([BL, 1], F32, tag="gsp")
    nc.scalar.activation(gsp[:], gse[:], AF.Ln, bias=1.0)
    gamma = small.tile([BL, 1], F32, tag="gamma")
    nc.vector.tensor_scalar(gamma[:], gsp[:], 1.0, None, op0=ALU.add)

    # ---------------- duplicate scalars to (2b+s) rows via PE ------------
    # scal_pack cols: bscale, gate, omg, gamma, sk0, sk1, sk2
    scal_pack = small.tile([BL, 7], F32, tag="scal_pack")
    nc.vector.tensor_copy(scal_pack[:, 0:1], bscale[:])
    nc.vector.tensor_copy(scal_pack[:, 1:2], g_t[:])
    nc.vector.tensor_copy(scal_pack[:, 2:3], omg[:])
    nc.vector.tensor_copy(scal_pack[:, 3:4], gamma[:])
    nc.vector.tensor_copy(scal_pack[:, 4:7], sk[:])
    scal2_ps_t = spsum.tile([P, 64], F32, tag="sm", name="scal2_ps_t")
    scal2_ps = scal2_ps_t[:, 0:7]
    nc.tensor.matmul(scal2_ps[:], d64_f[:], scal_pack[:], start=True, stop=True)
    scal2 = small.tile([P, 7], F32, tag="scal2")
    nc.vector.tensor_copy(scal2[:], scal2_ps[:])
    bscale2 = scal2[:, 0:1]
    g2 = scal2[:, 1:2]
    omg2 = scal2[:, 2:3]
    gamma2 = scal2[:, 3:4]
    sk1_2 = scal2[:, 6 - 1:6]          # sk1 at col 5

    # conv cross-term coefficients:
    # skU[2b] = sk2_b, skU[2b+1] = sk0_b (unshifted cross term)
    # skS[2b] = sk0_b (shift -1 on gE), skS[2b+1] = sk2_b (shift +1 on gO)
    skx = small.tile([BL, 2], F32, tag="skx")
    nc.vector.tensor_copy(skx[:, 0:1], sk[:, 2:3])
    nc.vector.tensor_copy(skx[:, 1:2], sk[:, 0:1])
    skx2 = small.tile([BL, 2], F32, tag="skx2")
    nc.vector.tensor_copy(skx2[:, 0:1], sk[:, 0:1])
    nc.vector.tensor_copy(skx2[:, 1:2], sk[:, 2:3])
    sk2ps_t = spsum.tile([P, 64], F32, tag="sm", name="sk2ps_t")
    sk2ps = sk2ps_t[:, 0:2]
    nc.tensor.matmul(sk2ps[:], de_f[:], skx[:], start=True, stop=False)
    nc.tensor.matmul(sk2ps[:], do_f[:], skx2[:], start=False, stop=True)
    skuv = small.tile([P, 2], F32, tag="skuv")
    nc.vector.tensor_copy(skuv[:], sk2ps[:])
    skU = skuv[:, 0:1]
    skS = skuv[:, 1:2]

    # q2 [128, 64] fp32: col b = [q_b ; q_b] (drain scale operand)
    qT_ps_t = spsum.tile([P, 64], F32, tag="sm", name="qT_ps_t")
    qT_ps = qT_ps_t[0:BL, 0:BL]
    nc.tensor.transpose(qT_ps[:], q_sb[:], ident[0:BL, 0:BL])
    q2 = small.tile([P, BL], F32, tag="q2")
    nc.vector.tensor_copy(q2[0:64, :], qT_ps[:])
    nc.vector.tensor_copy(q2[64:128, :], qT_ps[:])

    # ---------------- stage B ----------------
    d_dot = big.tile([P, MH2], F32, tag="ddot")
    d_nsq = big.tile([P, MH2], F32, tag="dnsq")
    dd16 = big.tile([P, MH2], F16, tag="dd16")     # exp'd numerators (fp16)
    prev16 = big.tile([P, MH2], F16, tag="prev16")
    cp16 = big.tile([P, MH2], F16, tag="cp16")     # conv(prev)
    cd16 = big.tile([P, MH2], F16, tag="cd16")     # conv(exp)
    gEp = big.tile([P, MH2], F16, tag="gEp")
    gOp = big.tile([P, MH2], F16, tag="gOp")
    gEd = big.tile([P, MH2], F16, tag="gEd")
    gOd = big.tile([P, MH2], F16, tag="gOd")
    nc.gpsimd.dma_start(prev16[:], prev2[:])       # cast fp32->fp16

    def vview(t, ph, vlo, vhi, plo, phi):
        # view t[:, 1024*ph + 128*v + p] for v in [vlo,vhi), p in [plo,phi)
        return t[:, 1024 * ph:1024 * (ph + 1)].rearrange(
            "r (v p) -> r v p", v=NV)[:, vlo:vhi, plo:phi]

    def conv_half_steps(src, gEt, gOt, dst, ph, nm):
        """Closure list: dst[hsl] = 3-tap circular conv of src for the ph
        half, except the 4 cross-half/circular single-column terms."""
        steps = []
        hsl = slice(1024 * ph, 1024 * (ph + 1))
        for ci, c0 in enumerate(range(1024 * ph, 1024 * (ph + 1), 512)):
            def swp(c0=c0, ci=ci):
                sp = tps.tile([P, 512], F32, tag="T", name=f"swpE_{nm}{ph}{ci}")
                nc.tensor.matmul(sp[:], swpe16[:], src[:, c0:c0 + 512],
                                 start=True, stop=True)
                nc.vector.tensor_copy(gEt[:, c0:c0 + 512], sp[:])
                sp2 = tps.tile([P, 512], F32, tag="T",
                               name=f"swpO_{nm}{ph}{ci}")
                nc.tensor.matmul(sp2[:], swpo16[:], src[:, c0:c0 + 512],
                                 start=True, stop=True)
                nc.vector.tensor_copy(gOt[:, c0:c0 + 512], sp2[:])
            steps.append(swp)
        steps.append(lambda: nc.vector.tensor_scalar(
            dst[:, hsl], src[:, hsl], sk1_2, None, op0=ALU.mult))
        steps.append(lambda: nc.vector.scalar_tensor_tensor(
            dst[:, hsl], gEt[:, hsl], skU, dst[:, hsl],
            op0=ALU.mult, op1=ALU.add))
        steps.append(lambda: nc.vector.scalar_tensor_tensor(
            dst[:, hsl], gOt[:, hsl], skU, dst[:, hsl],
            op0=ALU.mult, op1=ALU.add))
        # S+ shifted gE (src col j-1 -> dst col j)
        steps.append(lambda: nc.vector.scalar_tensor_tensor(
            vview(dst, ph, 1, NV, 0, P), vview(gEt, ph, 0, NV - 1, 0, P), skS,
            vview(dst, ph, 1, NV, 0, P), op0=ALU.mult, op1=ALU.add))
        steps.append(lambda: nc.vector.scalar_tensor_tensor(
            vview(dst, ph, 0, 1, 1, P), vview(gEt, ph, NV - 1, NV, 0, P - 1),
            skS, vview(dst, ph, 0, 1, 1, P), op0=ALU.mult, op1=ALU.add))
        # S- shifted gO (src col j+1 -> dst col j)
        steps.append(lambda: nc.vector.scalar_tensor_tensor(
            vview(dst, ph, 0, NV - 1, 0, P), vview(gOt, ph, 1, NV, 0, P), skS,
            vview(dst, ph, 0, NV - 1, 0, P), op0=ALU.mult, op1=ALU.add))
        steps.append(lambda: nc.vector.scalar_tensor_tensor(
            vview(dst, ph, NV - 1, NV, 0, P - 1), vview(gOt, ph, 0, 1, 1, P),
            skS, vview(dst, ph, NV - 1, NV, 0, P - 1),
            op0=ALU.mult, op1=ALU.add))
        return steps

    def emit_conv_fixups(gEt, gOt, dst):
        # ph boundary: dst j=1024 <- src j=1023 ; circular: dst j=0 <- j=2047
        nc.vector.scalar_tensor_tensor(dst[:, 1024:1025], gEt[:, 1023:1024],
                                       skS, dst[:, 1024:1025],
                                       op0=ALU.mult, op1=ALU.add)
        nc.vector.scalar_tensor_tensor(dst[:, 0:1], gEt[:, MH2 - 1:MH2], skS,
                                       dst[:, 0:1], op0=ALU.mult, op1=ALU.add)
        # mirror for gO: dst j=1023 <- j=1024 ; dst j=2047 <- j=0
        nc.vector.scalar_tensor_tensor(dst[:, 1023:1024], gOt[:, 1024:1025],
                                       skS, dst[:, 1023:1024],
                                       op0=ALU.mult, op1=ALU.add)
        nc.vector.scalar_tensor_tensor(dst[:, MH2 - 1:MH2], gOt[:, 0:1], skS,
                                       dst[:, MH2 - 1:MH2],
                                       op0=ALU.mult, op1=ALU.add)

    LAG = 6  # k-tiles the accumulation matmuls trail the transposes by
    tile_idx = 0
    bg = []  # background (overlap) work, pumped one step per few batches
    # L tiles are per-(ph,b) [128, (v s d)] fp16; partition line = one
    # contiguous 4KB fp32 run in HBM (m-rows 16p..16p+16 of the ph half).
    lv_all = mem[:].rearrange("b (ph p v s) d -> b ph p (v s d)",
                              ph=NPH, p=P, v=NV, s=2)

    for ph in range(NPH):
        dbank = [accps.tile([P, 512], F32, tag=f"dot{k}", name=f"dot{k}") for k in range(2)]
        nbank = [accps.tile([P, 512], F32, tag=f"nrm{k}", name=f"nrm{k}") for k in range(2)]

        def emit_mm(item):
            Ct, St, b_, k_ = item
            msl = master[:, 126 - 2 * b_:254 - 2 * b_]
            nc.tensor.matmul(dbank[k_][:], msl, Ct[:],
                             start=(b_ == 0), stop=(b_ == BL - 1),
                             skip_group_check=True)
            nc.tensor.matmul(nbank[k_][:], msl, St[:],
                             start=(b_ == 0), stop=(b_ == BL - 1),
                             skip_group_check=True)

        pend = []
        if ph == 0:
            # conv(prev) runs in the background during stage B
            bg.extend(conv_half_steps(prev16, gEp, gOp, cp16, 0, "p"))
            bg.extend(conv_half_steps(prev16, gEp, gOp, cp16, 1, "p"))
            bg.append(lambda: emit_conv_fixups(gEp, gOp, cp16))
        GRP = 8   # batches per cast-DMA (amortizes SWDGE overhead)
        for b in range(BL):
            if b % 3 == 2 and bg:
                bg.pop(0)()
            if b % GRP == 0:
                Lg = mem_pool.tile([P, GRP * 1024], F16, tag="L")
                nc.gpsimd.dma_start(
                    Lg[:].rearrange("p (g f) -> p g f", g=GRP),
                    lv_all[b:b + GRP, ph].rearrange("g p f -> p g f"))
            L = Lg[:, (b % GRP) * 1024:(b % GRP + 1) * 1024]

            for k in range(2):
                T = tps.tile([P, 512], F16, tag="T")
                for tt in range(4):
                    v = 4 * k + tt
                    nc.tensor.transpose(T[:, tt * P:(tt + 1) * P],
                                        L[:, v * P:(v + 1) * P], ident16[:])
                Ct = cs_pool.tile([P, 512], F16, tag="C")
                St = ss_pool.tile([P, 512], F16, tag="S")
                nc.vector.tensor_scalar(Ct[:], T[:], q2[:, b:b + 1], None,
                                        op0=ALU.mult)
                if tile_idx % ST_MOD < ST_DVE:
                    nc.vector.tensor_tensor(St[:], T[:], T[:], op=ALU.mult)
                else:
                    nc.scalar.activation(St[:], T[:], AF.Square)
                pend.append((Ct, St, b, k))
                if len(pend) > LAG:
                    emit_mm(pend.pop(0))
                tile_idx += 1
        for item in pend:
            emit_mm(item)
        pend = []

        for k in range(2):
            c0 = 1024 * ph + 512 * k
            nc.vector.tensor_copy(d_dot[:, c0:c0 + 512], dbank[k][:])
            nc.scalar.activation(d_nsq[:, c0:c0 + 512], nbank[k][:], AF.Copy)

        # ---------------- stage C on this m-half as soon as it drains ----
        h0 = 1024 * ph
        dd = d_dot[:, h0:h0 + 1024]
        dn = d_nsq[:, h0:h0 + 1024]
        # sim = dot * rsqrt(nsq) * bscale ; softmax numerator exp(beta*cos)
        nc.scalar.activation(dn[:], dn[:], AF.Ln)
        nc.scalar.activation(dn[:], dn[:], AF.Exp, scale=-0.5)
        nc.vector.tensor_tensor(dd[:], dd[:], dn[:], op=ALU.mult)
        nc.scalar.activation(dd16[:, h0:h0 + 1024], dd[:], AF.Exp,
                             scale=bscale2)
        # conv(exp) for this half: background during ph=1 / direct at tail
        if ph == 0:
            bg.extend(conv_half_steps(dd16, gEd, gOd, cd16, 0, "d"))
        else:
            for step in conv_half_steps(dd16, gEd, gOd, cd16, 1, "d"):
                step()

    # ---------------- stage C tail: softmax norm, gate, conv, sharpen ----
    while bg:
        bg.pop(0)()
    emit_conv_fixups(gEd, gOd, cd16)
    esum_h = small.tile([P, 2], F32, tag="esum_h")
    for ph in range(NPH):
        nc.vector.tensor_reduce(esum_h[:, ph:ph + 1],
                                dd16[:, 1024 * ph:1024 * (ph + 1)],
                                axis=AX.X, op=ALU.add)
    esum = small.tile([P, 1], F32, tag="esum")
    nc.vector.tensor_reduce(esum[:], esum_h[:], axis=AX.X, op=ALU.add)
    eps_ps_t = spsum.tile([P, 64], F32, tag="sm", name="eps_ps_t")
    eps_ps = eps_ps_t[0:BL, 0:1]
    nc.tensor.matmul(eps_ps[:], d64t[:], esum[:], start=True, stop=True)
    erec64 = small.tile([BL, 1], F32, tag="erec64")
    nc.vector.reciprocal(erec64[:], eps_ps[:])
    er_ps_t = spsum.tile([P, 64], F32, tag="sm", name="er_ps_t")
    er_ps = er_ps_t[:, 0:1]
    nc.tensor.matmul(er_ps[:], d64_f[:], erec64[:], start=True, stop=True)
    erec2 = small.tile([P, 1], F32, tag="erec2")
    nc.vector.tensor_copy(erec2[:], er_ps[:])
    galpha2 = small.tile([P, 1], F32, tag="galpha2")
    nc.vector.tensor_tensor(galpha2[:], g2, erec2[:], op=ALU.mult)

    # combine: conv(gated) = galpha*conv(exp) + omg*conv(prev), then
    # sharpen (cv+eps)^gamma and normalize.  sh reuses d_nsq (fp32 scratch).
    t16 = big.tile([P, MH2], F16, tag="t16")
    nc.vector.tensor_scalar(t16[:], cd16[:], galpha2[:, 0:1], None,
                            op0=ALU.mult)
    nc.vector.scalar_tensor_tensor(cp16[:], cp16[:], omg2, t16[:],
                                   op0=ALU.mult, op1=ALU.add)
    sh = d_nsq
    nc.scalar.activation(sh[:], cp16[:], AF.Ln, bias=EPS)
    nc.scalar.activation(sh[:], sh[:], AF.Exp, scale=gamma2)
    psumt = small.tile([P, 1], F32, tag="psumt")
    nc.vector.tensor_reduce(psumt[:], sh[:], axis=AX.X, op=ALU.add)
    pp_ps_t = spsum.tile([P, 64], F32, tag="sm", name="pp_ps_t")
    pp_ps = pp_ps_t[0:BL, 0:1]
    nc.tensor.matmul(pp_ps[:], d64t[:], psumt[:], start=True, stop=True)
    ppe = small.tile([BL, 1], F32, tag="ppe")
    nc.vector.tensor_scalar(ppe[:], pp_ps[:], EPS, None, op0=ALU.add)
    prc64 = small.tile([BL, 1], F32, tag="prc64")
    nc.vector.reciprocal(prc64[:], ppe[:])
    pr_ps_t = spsum.tile([P, 64], F32, tag="sm", name="pr_ps_t")
    pr_ps = pr_ps_t[:, 0:1]
    nc.tensor.matmul(pr_ps[:], d64_f[:], prc64[:], start=True, stop=True)
    prc2 = small.tile([P, 1], F32, tag="prc2")
    nc.vector.tensor_copy(prc2[:], pr_ps[:])

    nc.scalar.activation(sh[:], sh[:], AF.Copy, scale=prc2[:])
    nc.sync.dma_start(out[:], sh[:])


def build(split_waits=True):
    nc = bass.Bass()
    mem = nc.dram_tensor("memory", [BL, M, D], F32, kind="ExternalInput")
    cs = nc.dram_tensor("controller_state", [BL, C], F32, kind="ExternalInput")
    prev2 = nc.dram_tensor("prev2", [P, MH2], F32, kind="ExternalInput")
    wk_pack = nc.dram_tensor("wk_pack", [P, 2 * D], F32, kind="ExternalInput")
    wh_pack = nc.dram_tensor("wh_pack", [P, 12], F32, kind="ExternalInput")
    b_pack = nc.dram_tensor("b_pack", [1, 6], F32, kind="ExternalInput")
    out = nc.dram_tensor("out", [P, MH2], F32, kind="ExternalOutput")

    eps_t = nc.alloc_sbuf_tensor("const-f32-eps", [128, 1], F32)
    nc.gpsimd.memset(eps_t.ap(), EPS)
    nc.const_aps.aps[(F32, EPS)] = eps_t.ap()
    nc.all_engine_barrier()

    with tile.TileContext(nc) as tc:
        with ExitStack() as ctx:
            tc._body_ctx = ctx
            _body(tc, nc, mem, cs, prev2, wk_pack, b_pack, wh_pack, out)
    if split_waits:
        _split_multiwait(nc)
    return nc


def _split_multiwait(nc, max_waits=1):
    """Walrus ISA structs encode a limited number of semaphore waits per
    instruction. Move all but one wait of any multi-wait instruction onto
    same-engine InstNoOp instructions inserted directly before it."""
    for fn in nc.m.functions:
        for blk in fn.blocks:
            insts = blk.instructions
            idx = 0
            while idx < len(insts):
                inst = insts[idx]
                si = inst.sync_info
                if si is not None and len(si.on_wait) > max_waits:
                    waits = list(si.on_wait)
                    extra, keep = waits[:-max_waits], waits[-max_waits:]
                    for w in extra:
                        nop = mybir.InstNoOp(
                            name=nc.get_next_instruction_name(),
                            sync_info=mybir.SyncInfo(on_wait=[w], on_update=[]),
                            bass_nofuse=True,
                            engine=inst.engine,
                        )
                        insts.insert(idx, nop)
                        idx += 1
                    inst.sync_info = mybir.SyncInfo(
                        on_wait=keep, on_update=list(si.on_update))
                idx += 1


_NC = None


def _get_nc():
    global _NC
    if _NC is None:
        _NC = build()
    return _NC


def _j_of_c():
    # j = 1024*ph + 8*p + v for c = 1024*ph + 128*v + p
    c = np.arange(MH2)
    ph = c // 1024
    v = (c % 1024) // P
    p = c % P
    return 1024 * ph + 8 * p + v


_JC = _j_of_c()


def _make_in_maps(inputs):
    full = {k: np.ascontiguousarray(np.asarray(v, dtype=np.float32))
            for k, v in inputs.items()}
    wk_pack = np.ascontiguousarray(
        np.concatenate([full["Wk"][0:P, :], full["Wk"][P:C, :]], axis=1))
    wh = np.concatenate(
        [full["Wb"], full["Wgate"], full["Ws"], full["Wg"]], axis=1)
    wh_pack = np.ascontiguousarray(np.concatenate([wh[0:P], wh[P:C]], axis=1))
    b_pack = np.ascontiguousarray(np.concatenate(
        [full["bb"].reshape(-1), full["bgate"].reshape(-1),
         full["bs"].reshape(-1), full["bg"].reshape(-1)]).reshape(1, 6))
    in_maps = []
    for c in range(NCORES):
        sl = slice(c * BL, (c + 1) * BL)
        # prev2 rows 2b+s = prev[b, 2*j(c)+s]
        pv = full["previous_weights"][sl]            # [64, 4096]
        pj = pv.reshape(BL, MH2, 2)[:, _JC, :]       # [64, 2048(c), 2(s)]
        prev2 = np.ascontiguousarray(
            pj.transpose(0, 2, 1).reshape(P, MH2))
        in_maps.append({
            "memory": full["memory"][sl],
            "controller_state": full["controller_state"][sl],
            "prev2": prev2,
            "wk_pack": wk_pack, "wh_pack": wh_pack, "b_pack": b_pack,
        })
    return in_maps


def run(inputs, **kwargs):
    from concourse.bass_utils import run_bass_kernel_spmd
    nc = _get_nc()
    res = run_bass_kernel_spmd(nc, _make_in_maps(inputs),
                               list(range(NCORES)), **kwargs)
    inv = np.argsort(_JC)
    outs = []
    for c in range(NCORES):
        o2 = res.results[c]["out"]                   # [128, 2048] rows 2b+s
        o = o2.reshape(BL, 2, MH2)[:, :, inv]        # cols back to j-order
        o = o.transpose(0, 2, 1).reshape(BL, M)      # m = 2j+s
        outs.append(o)
    out = np.concatenate(outs, axis=0)
    return out.astype(np.float32), res


def kernel(**inputs):
    out, _ = run(inputs)
    return out


# revision 19
# speedup vs baseline: 1.0362x; 1.0038x over previous
"""Trainium2 Bass kernel: DifferentiableAddressingHead (NTM-style addressing).

V3: fp16 big-data path. Data parallel over batch (64 rows/core).

Per-core dataflow:
  m-mapping: m = 2048*ph + 16*p + 2*v + s  (ph: m-half, p: SBUF partition,
  v: 8 sub-rows, s: parity). Stage-C layout: rows r = 2b+s, cols
  c = 1024*ph + 128*v + p  <->  j = m//2 = 1024*ph + 8*p + v.
  Host interleaves prev_weights / de-interleaves the output.

  Big path (per b):
    one SWDGE cast-DMA mem[b] fp32 -> L [128, (ph v s d)] fp16
    (4KB contiguous runs per partition per ph).
    PE: 16 fp16 transposes [128,128] -> T psum fp16 [(s,d), p-cols]
    DVE: Ct = q-scaled copy of T (fp16, 2x from fp16 psum)
    DVE/ACT: St = Square(T) (fp16)
    PE: fp16 accumulate-MMs vs shared ones-master (cols 126/127 slide),
        rows 2b|2b+1 of dots/norms psum banks, start b==0 stop b==63.
  Stage C: cosine sim + softmax + gate + 3-tap circular conv (cross-parity
  via PE permutation matmuls SWP_E/SWP_O; j-shifts via strided v/p APs) +
  sharpen + normalize, in fp32, rows (2b+s).
"""

from contextlib import ExitStack

import numpy as np

import concourse.bass as bass
import concourse.tile as tile
from concourse import masks, mybir

B, M, D, C = 512, 4096, 64, 256
NCORES = 8
BL = B // NCORES          # 64 batch rows per core
NSHIFT = 3
EPS = 1e-8

F32 = mybir.dt.float32
F16 = mybir.dt.float16
AF = mybir.ActivationFunctionType
ALU = mybir.AluOpType
AX = mybir.AxisListType

P = 128
MH2 = M // 2              # 2048 stage-C columns
NPH = 2                   # m-half phases
NV = 8                    # v sub-rows per (ph, partition)

# fraction of St squares routed to DVE instead of ACT (engine balance):
# tile t goes to DVE when t % ST_MOD < ST_DVE
ST_MOD = 4
ST_DVE = 0   # TensorTensor cannot read 2 PSUM inputs; St stays on ACT


def _body(tc, nc, mem, cs, prev2, Wk, bb, wheads, out):
    ctx = tc._body_ctx

    const = ctx.enter_context(tc.tile_pool(name="const", bufs=1))
    small = ctx.enter_context(tc.tile_pool(name="small", bufs=1))
    spsum = ctx.enter_context(tc.tile_pool(name="spsum", bufs=1, space="PSUM"))
    accps = ctx.enter_context(tc.tile_pool(name="accps", bufs=1, space="PSUM"))
    tps = ctx.enter_context(tc.tile_pool(name="tps", bufs=3, space="PSUM"))
    mem_pool = ctx.enter_context(tc.tile_pool(name="mem", bufs=3))
    cs_pool = ctx.enter_context(tc.tile_pool(name="cspool", bufs=10))
    ss_pool = ctx.enter_context(tc.tile_pool(name="sspool", bufs=10))
    big = ctx.enter_context(tc.tile_pool(name="big", bufs=1))

    # ---------------- constants ----------------
    ident = const.tile([P, P], F32, tag="ident")
    masks.make_identity(nc, ident[:])
    ident16 = const.tile([P, P], F16, tag="ident16")
    nc.vector.tensor_copy(ident16[:], ident[:])
    ones_row = const.tile([1, P], F32, tag="ones")
    nc.gpsimd.memset(ones_row[:], 1.0)

    # ones-master for selector matmuls (fp16): col 126 = ones rows 0:64,
    # col 127 = ones rows 64:128; batch b uses slice [126-2b : 254-2b]
    master = const.tile([P, 2 * P], F16, tag="master")
    nc.vector.memset(master[:], 0.0)
    nc.vector.memset(master[0:64, 126:127], 1.0)
    nc.vector.memset(master[64:128, 127:128], 1.0)

    # D64 family: DE_f[b, 2b] = 1, DO_f[b, 2b+1] = 1, D64_f = DE + DO
    de_f = const.tile([BL, P], F32, tag="de_f")
    do_f = const.tile([BL, P], F32, tag="do_f")
    d64_f = const.tile([BL, P], F32, tag="d64_f")
    nc.vector.memset(de_f[:], 0.0)
    nc.vector.memset(do_f[:], 0.0)
    i64 = ident[0:BL, 0:BL]
    nc.vector.tensor_copy(
        de_f[:].rearrange("b (c two) -> b c two", two=2)[:, :, 0], i64)
    nc.vector.tensor_copy(
        do_f[:].rearrange("b (c two) -> b c two", two=2)[:, :, 1], i64)
    nc.vector.tensor_tensor(d64_f[:], de_f[:], do_f[:], op=ALU.add)
    # D64T [128, 64]: col b = ones at rows 2b, 2b+1 (for pair-sums)
    d64t_ps_t = spsum.tile([P, 64], F32, tag="sm", name="d64t_ps_t")
    d64t_ps = d64t_ps_t[:, 0:BL]
    nc.tensor.transpose(d64t_ps[:], d64_f[:], ident[0:BL, 0:BL])
    d64t = const.tile([P, BL], F32, tag="d64t")
    nc.vector.tensor_copy(d64t[:], d64t_ps[:])

    # parity-swap permutations (free-dim shifted identity halves):
    # gE = SWP_E^T @ g puts g[odd rows] onto even rows (zeros on odd), and
    # gO = SWP_O^T @ g puts g[even rows] onto odd rows.
    swpe = const.tile([P, P], F32, tag="swpe")
    swpo = const.tile([P, P], F32, tag="swpo")
    nc.vector.memset(swpe[:], 0.0)
    nc.vector.memset(swpo[:], 0.0)
    iv = ident[:].rearrange("p (c two) -> p c two", two=2)
    nc.vector.tensor_copy(
        swpe[:].rearrange("p (c two) -> p c two", two=2)[:, :, 0], iv[:, :, 1])
    nc.vector.tensor_copy(
        swpo[:].rearrange("p (c two) -> p c two", two=2)[:, :, 1], iv[:, :, 0])
    swpe16 = const.tile([P, P], F16, tag="swpe16")
    swpo16 = const.tile([P, P], F16, tag="swpo16")
    nc.vector.tensor_copy(swpe16[:], swpe[:])
    nc.vector.tensor_copy(swpo16[:], swpo[:])

    # ---------------- load controller + host-prepacked weights ----------
    cs_sb = small.tile([BL, C], F32, tag="cs")
    nc.sync.dma_start(cs_sb[:], cs[:])
    wk_sb = small.tile([P, 2 * D], F32, tag="wk")
    nc.sync.dma_start(wk_sb[:], Wk[:])           # wk_pack [128, 128]
    wh_sb = small.tile([P, 12], F32, tag="wh")
    nc.sync.dma_start(wh_sb[:], wheads[:])       # wh_pack [128, 12]
    brow = small.tile([1, 6], F32, tag="brow")
    nc.sync.dma_start(brow[:], bb[:])            # b_pack [1, 6]

    # ---------------- transpose cs -> csT [C(2x128 part), BL] --------------
    csT = small.tile([P, 2 * BL], F32, tag="csT")
    for ci in range(2):
        t_ps_t = spsum.tile([P, 64], F32, tag="sm", name="t_ps_t")
        t_ps = t_ps_t[:, 0:BL]
        nc.tensor.transpose(t_ps[:], cs_sb[:, ci * P:(ci + 1) * P], ident[0:BL, 0:BL])
        nc.vector.tensor_copy(csT[:, ci * BL:(ci + 1) * BL], t_ps[:])

    # ---------------- query + heads on PE ----------------
    q_ps_t = spsum.tile([P, 64], F32, tag="sm", name="q_ps_t")
    q_ps = q_ps_t[0:BL, 0:D]
    nc.tensor.matmul(q_ps[:], csT[:, 0:BL], wk_sb[:, 0:D], start=True, stop=False)
    nc.tensor.matmul(q_ps[:], csT[:, BL:2 * BL], wk_sb[:, D:2 * D],
                     start=False, stop=True)
    q_sb = small.tile([BL, D], F32, tag="qsb")
    nc.vector.tensor_copy(q_sb[:], q_ps[:])

    h_ps_t = spsum.tile([P, 64], F32, tag="sm", name="h_ps_t")
    h_ps = h_ps_t[0:BL, 0:6]
    nc.tensor.matmul(h_ps[:], csT[:, 0:BL], wh_sb[:, 0:6], start=True, stop=False)
    nc.tensor.matmul(h_ps[:], csT[:, BL:2 * BL], wh_sb[:, 6:12],
                     start=False, stop=False)
    nc.tensor.matmul(h_ps[:], ones_row[0:1, 0:BL], brow[:], start=False, stop=True)
    h_sb = small.tile([BL, 6], F32, tag="hsb")
    nc.vector.tensor_copy(h_sb[:], h_ps[:])

    # ---------------- per-batch scalars ([BL, 1]) ----------------
    qsq = small.tile([BL, D], F32, tag="qsq")
    qn2 = small.tile([BL, 1], F32, tag="qn2")
    nc.scalar.activation(qsq[:], q_sb[:], AF.Square, accum_out=qn2[:])
    qnorm = small.tile([BL, 1], F32, tag="qnorm")
    nc.scalar.activation(qnorm[:], qn2[:], AF.Sqrt)
    qne = small.tile([BL, 1], F32, tag="qne")
    nc.vector.tensor_scalar(qne[:], qnorm[:], EPS, None, op0=ALU.add)
    qrecip = small.tile([BL, 1], F32, tag="qrecip")
    nc.vector.reciprocal(qrecip[:], qne[:])

    # bscale = (softplus(h0)+1) / (|q|+eps)
    spe = small.tile([BL, 1], F32, tag="spe")
    nc.scalar.activation(spe[:], h_sb[:, 0:1], AF.Exp)
    spb = small.tile([BL, 1], F32, tag="spb")
    nc.scalar.activation(spb[:], spe[:], AF.Ln, bias=1.0)
    bscale = small.tile([BL, 1], F32, tag="bscale")
    nc.vector.tensor_scalar(bscale[:], spb[:], 1.0, qrecip[:],
                            op0=ALU.add, op1=ALU.mult)

    g_t = small.tile([BL, 1], F32, tag="gate")
    nc.scalar.activation(g_t[:], h_sb[:, 1:2], AF.Sigmoid)
    omg = small.tile([BL, 1], F32, tag="omg")
    nc.scalar.activation(omg[:], g_t[:], AF.Copy, bias=1.0, scale=-1.0)

    e3 = small.tile([BL, NSHIFT], F32, tag="e3")
    nc.scalar.activation(e3[:], h_sb[:, 2:5], AF.Exp)
    ssum = small.tile([BL, 1], F32, tag="ssum")
    nc.vector.tensor_reduce(ssum[:], e3[:], axis=AX.X, op=ALU.add)
    srec = small.tile([BL, 1], F32, tag="srec")
    nc.vector.reciprocal(srec[:], ssum[:])
    sk = small.tile([BL, NSHIFT], F32, tag="sk")
    nc.vector.tensor_scalar(sk[:], e3[:], srec[:], None, op0=ALU.mult)

    gse = small.tile([BL, 1], F32, tag="gse")
    nc.scalar.activation(gse[:], h_sb[:, 5:6], AF.Exp)
    gsp = small.tile([BL, 1], F32, tag="gsp")
    nc.scalar.activation(gsp[:], gse[:], AF.Ln, bias=1.0)
    gamma = small.tile([BL, 1], F32, tag="gamma")
    nc.vector.tensor_scalar(gamma[:], gsp[:], 1.0, None, op0=ALU.add)

    # ---------------- duplicate scalars to (2b+s) rows via PE ------------
    # scal_pack cols: bscale, gate, omg, gamma, sk0, sk1, sk2
    scal_pack = small.tile([BL, 7], F32, tag="scal_pack")
    nc.vector.tensor_copy(scal_pack[:, 0:1], bscale[:])
    nc.vector.tensor_copy(scal_pack[:, 1:2], g_t[:])
    nc.vector.tensor_copy(scal_pack[:, 2:3], omg[:])
    nc.vector.tensor_copy(scal_pack[:, 3:4], gamma[:])
    nc.vector.tensor_copy(scal_pack[:, 4:7], sk[:])
    scal2_ps_t = spsum.tile([P, 64], F32, tag="sm", name="scal2_ps_t")
    scal2_ps = scal2_ps_t[:, 0:7]
    nc.tensor.matmul(scal2_ps[:], d64_f[:], scal_pack[:], start=True, stop=True)
    scal2 = small.tile([P, 7], F32, tag="scal2")
    nc.vector.tensor_copy(scal2[:], scal2_ps[:])
    bscale2 = scal2[:, 0:1]
    g2 = scal2[:, 1:2]
    omg2 = scal2[:, 2:3]
    gamma2 = scal2[:, 3:4]
    sk1_2 = scal2[:, 6 - 1:6]          # sk1 at col 5

    # conv cross-term coefficients:
    # skU[2b] = sk2_b, skU[2b+1] = sk0_b (unshifted cross term)
    # skS[2b] = sk0_b (shift -1 on gE), skS[2b+1] = sk2_b (shift +1 on gO)
    skx = small.tile([BL, 2], F32, tag="skx")
    nc.vector.tensor_copy(skx[:, 0:1], sk[:, 2:3])
    nc.vector.tensor_copy(skx[:, 1:2], sk[:, 0:1])
    skx2 = small.tile([BL, 2], F32, tag="skx2")
    nc.vector.tensor_copy(skx2[:, 0:1], sk[:, 0:1])
    nc.vector.tensor_copy(skx2[:, 1:2], sk[:, 2:3])
    sk2ps_t = spsum.tile([P, 64], F32, tag="sm", name="sk2ps_t")
    sk2ps = sk2ps_t[:, 0:2]
    nc.tensor.matmul(sk2ps[:], de_f[:], skx[:], start=True, stop=False)
    nc.tensor.matmul(sk2ps[:], do_f[:], skx2[:], start=False, stop=True)
    skuv = small.tile([P, 2], F32, tag="skuv")
    nc.vector.tensor_copy(skuv[:], sk2ps[:])
    skU = skuv[:, 0:1]
    skS = skuv[:, 1:2]

    # q2 [128, 64] fp32: col b = [q_b ; q_b] (drain scale operand)
    qT_ps_t = spsum.tile([P, 64], F32, tag="sm", name="qT_ps_t")
    qT_ps = qT_ps_t[0:BL, 0:BL]
    nc.tensor.transpose(qT_ps[:], q_sb[:], ident[0:BL, 0:BL])
    q2 = small.tile([P, BL], F32, tag="q2")
    nc.vector.tensor_copy(q2[0:64, :], qT_ps[:])
    nc.vector.tensor_copy(q2[64:128, :], qT_ps[:])

    # ---------------- stage B ----------------
    d_dot = big.tile([P, MH2], F32, tag="ddot")
    d_nsq = big.tile([P, MH2], F32, tag="dnsq")
    dd16 = big.tile([P, MH2], F16, tag="dd16")     # exp'd numerators (fp16)
    prev16 = big.tile([P, MH2], F16, tag="prev16")
    cd16 = big.tile([P, MH2], F16, tag="cd16")     # conv(gated)
    gEd = big.tile([P, MH2], F16, tag="gEd")
    gOd = big.tile([P, MH2], F16, tag="gOd")
    nc.gpsimd.dma_start(prev16[:], prev2[:])       # cast fp32->fp16

    def vview(t, ph, vlo, vhi, plo, phi):
        # view t[:, 1024*ph + 128*v + p] for v in [vlo,vhi), p in [plo,phi)
        return t[:, 1024 * ph:1024 * (ph + 1)].rearrange(
            "r (v p) -> r v p", v=NV)[:, vlo:vhi, plo:phi]

    def conv_half_steps(src, gEt, gOt, dst, ph, nm):
        """Closure list: dst[hsl] = 3-tap circular conv of src for the ph
        half, except the 4 cross-half/circular single-column terms."""
        steps = []
        hsl = slice(1024 * ph, 1024 * (ph + 1))
        for ci, c0 in enumerate(range(1024 * ph, 1024 * (ph + 1), 512)):
            def swp(c0=c0, ci=ci):
                sp = tps.tile([P, 512], F32, tag="T", name=f"swpE_{nm}{ph}{ci}")
                nc.tensor.matmul(sp[:], swpe16[:], src[:, c0:c0 + 512],
                                 start=True, stop=True)
                nc.vector.tensor_copy(gEt[:, c0:c0 + 512], sp[:])
                sp2 = tps.tile([P, 512], F32, tag="T",
                               name=f"swpO_{nm}{ph}{ci}")
                nc.tensor.matmul(sp2[:], swpo16[:], src[:, c0:c0 + 512],
                                 start=True, stop=True)
                nc.vector.tensor_copy(gOt[:, c0:c0 + 512], sp2[:])
            steps.append(swp)
        steps.append(lambda: nc.vector.tensor_scalar(
            dst[:, hsl], src[:, hsl], sk1_2, None, op0=ALU.mult))
        steps.append(lambda: nc.vector.scalar_tensor_tensor(
            dst[:, hsl], gEt[:, hsl], skU, dst[:, hsl],
            op0=ALU.mult, op1=ALU.add))
        steps.append(lambda: nc.vector.scalar_tensor_tensor(
            dst[:, hsl], gOt[:, hsl], skU, dst[:, hsl],
            op0=ALU.mult, op1=ALU.add))
        # S+ shifted gE (src col j-1 -> dst col j)
        steps.append(lambda: nc.vector.scalar_tensor_tensor(
            vview(dst, ph, 1, NV, 0, P), vview(gEt, ph, 0, NV - 1, 0, P), skS,
            vview(dst, ph, 1, NV, 0, P), op0=ALU.mult, op1=ALU.add))
        steps.append(lambda: nc.vector.scalar_tensor_tensor(
            vview(dst, ph, 0, 1, 1, P), vview(gEt, ph, NV - 1, NV, 0, P - 1),
            skS, vview(dst, ph, 0, 1, 1, P), op0=ALU.mult, op1=ALU.add))
        # S- shifted gO (src col j+1 -> dst col j)
        steps.append(lambda: nc.vector.scalar_tensor_tensor(
            vview(dst, ph, 0, NV - 1, 0, P), vview(gOt, ph, 1, NV, 0, P), skS,
            vview(dst, ph, 0, NV - 1, 0, P), op0=ALU.mult, op1=ALU.add))
        steps.append(lambda: nc.vector.scalar_tensor_tensor(
            vview(dst, ph, NV - 1, NV, 0, P - 1), vview(gOt, ph, 0, 1, 1, P),
            skS, vview(dst, ph, NV - 1, NV, 0, P - 1),
            op0=ALU.mult, op1=ALU.add))
        return steps

    def emit_conv_fixups(gEt, gOt, dst):
        # ph boundary: dst j=1024 <- src j=1023 ; circular: dst j=0 <- j=2047
        nc.vector.scalar_tensor_tensor(dst[:, 1024:1025], gEt[:, 1023:1024],
                                       skS, dst[:, 1024:1025],
                                       op0=ALU.mult, op1=ALU.add)
        nc.vector.scalar_tensor_tensor(dst[:, 0:1], gEt[:, MH2 - 1:MH2], skS,
                                       dst[:, 0:1], op0=ALU.mult, op1=ALU.add)
        # mirror for gO: dst j=1023 <- j=1024 ; dst j=2047 <- j=0
        nc.vector.scalar_tensor_tensor(dst[:, 1023:1024], gOt[:, 1024:1025],
                                       skS, dst[:, 1023:1024],
                                       op0=ALU.mult, op1=ALU.add)
        nc.vector.scalar_tensor_tensor(dst[:, MH2 - 1:MH2], gOt[:, 0:1], skS,
                                       dst[:, MH2 - 1:MH2],
                                       op0=ALU.mult, op1=ALU.add)

    LAG = 6  # k-tiles the accumulation matmuls trail the transposes by
    tile_idx = 0
    # L tiles are per-(ph,b) [128, (v s d)] fp16; partition line = one
    # contiguous 4KB fp32 run in HBM (m-rows 16p..16p+16 of the ph half).
    lv_all = mem[:].rearrange("b (ph p v s) d -> b ph p (v s d)",
                              ph=NPH, p=P, v=NV, s=2)

    for ph in range(NPH):
        dbank = [accps.tile([P, 512], F32, tag=f"dot{k}", name=f"dot{k}") for k in range(2)]
        nbank = [accps.tile([P, 512], F32, tag=f"nrm{k}", name=f"nrm{k}") for k in range(2)]

        def emit_mm(item):
            Ct, St, b_, k_ = item
            msl = master[:, 126 - 2 * b_:254 - 2 * b_]
            nc.tensor.matmul(dbank[k_][:], msl, Ct[:],
                             start=(b_ == 0), stop=(b_ == BL - 1),
                             skip_group_check=True)
            nc.tensor.matmul(nbank[k_][:], msl, St[:],
                             start=(b_ == 0), stop=(b_ == BL - 1),
                             skip_group_check=True)

        pend = []
        GRP = 8   # batches per cast-DMA (amortizes SWDGE overhead)
        for b in range(BL):
            if b % GRP == 0:
                Lg = mem_pool.tile([P, GRP * 1024], F16, tag="L")
                nc.gpsimd.dma_start(
                    Lg[:].rearrange("p (g f) -> p g f", g=GRP),
                    lv_all[b:b + GRP, ph].rearrange("g p f -> p g f"))
            L = Lg[:, (b % GRP) * 1024:(b % GRP + 1) * 1024]

            for k in range(2):
                T = tps.tile([P, 512], F16, tag="T")
                for tt in range(4):
                    v = 4 * k + tt
                    nc.tensor.transpose(T[:, tt * P:(tt + 1) * P],
                                        L[:, v * P:(v + 1) * P], ident16[:])
                Ct = cs_pool.tile([P, 512], F16, tag="C")
                St = ss_pool.tile([P, 512], F16, tag="S")
                nc.vector.tensor_scalar(Ct[:], T[:], q2[:, b:b + 1], None,
                                        op0=ALU.mult)
                if tile_idx % ST_MOD < ST_DVE:
                    nc.vector.tensor_tensor(St[:], T[:], T[:], op=ALU.mult)
                else:
                    nc.scalar.activation(St[:], T[:], AF.Square)
                pend.append((Ct, St, b, k))
                if len(pend) > LAG:
                    emit_mm(pend.pop(0))
                tile_idx += 1
        for item in pend:
            emit_mm(item)
        pend = []

        for k in range(2):
            c0 = 1024 * ph + 512 * k
            nc.vector.tensor_copy(d_dot[:, c0:c0 + 512], dbank[k][:])
            nc.scalar.activation(d_nsq[:, c0:c0 + 512], nbank[k][:], AF.Copy)

        # ---------------- stage C on this m-half as soon as it drains ----
        h0 = 1024 * ph
        dd = d_dot[:, h0:h0 + 1024]
        dn = d_nsq[:, h0:h0 + 1024]
        # sim = dot * rsqrt(nsq) * bscale ; softmax numerator exp(beta*cos)
        nc.scalar.activation(dn[:], dn[:], AF.Ln)
        nc.scalar.activation(dn[:], dn[:], AF.Exp, scale=-0.5)
        nc.vector.tensor_tensor(dd[:], dd[:], dn[:], op=ALU.mult)
        nc.scalar.activation(dd16[:, h0:h0 + 1024], dd[:], AF.Exp,
                             scale=bscale2)

    # ---------------- stage C tail: softmax norm, gate, conv, sharpen ----
    esum_h = small.tile([P, 2], F32, tag="esum_h")
    for ph in range(NPH):
        nc.vector.tensor_reduce(esum_h[:, ph:ph + 1],
                                dd16[:, 1024 * ph:1024 * (ph + 1)],
                                axis=AX.X, op=ALU.add)
    esum = small.tile([P, 1], F32, tag="esum")
    nc.vector.tensor_reduce(esum[:], esum_h[:], axis=AX.X, op=ALU.add)
    eps_ps_t = spsum.tile([P, 64], F32, tag="sm", name="eps_ps_t")
    eps_ps = eps_ps_t[0:BL, 0:1]
    nc.tensor.matmul(eps_ps[:], d64t[:], esum[:], start=True, stop=True)
    erec64 = small.tile([BL, 1], F32, tag="erec64")
    nc.vector.reciprocal(erec64[:], eps_ps[:])
    er_ps_t = spsum.tile([P, 64], F32, tag="sm", name="er_ps_t")
    er_ps = er_ps_t[:, 0:1]
    nc.tensor.matmul(er_ps[:], d64_f[:], erec64[:], start=True, stop=True)
    erec2 = small.tile([P, 1], F32, tag="erec2")
    nc.vector.tensor_copy(erec2[:], er_ps[:])
    galpha2 = small.tile([P, 1], F32, tag="galpha2")
    nc.vector.tensor_tensor(galpha2[:], g2, erec2[:], op=ALU.mult)

    # gated (fp16): ga = galpha*exp + omg*prev, then 3-tap conv + sharpen.
    ga16 = big.tile([P, MH2], F16, tag="ga16")
    for ph in range(NPH):
        hsl = slice(1024 * ph, 1024 * (ph + 1))
        nc.scalar.activation(ga16[:, hsl], dd16[:, hsl], AF.Copy,
                             scale=galpha2[:])
    nc.vector.scalar_tensor_tensor(ga16[:], prev16[:], omg2, ga16[:],
                                   op0=ALU.mult, op1=ALU.add)
    for ph in range(NPH):
        for step in conv_half_steps(ga16, gEd, gOd, cd16, ph, "g"):
            step()
    emit_conv_fixups(gEd, gOd, cd16)
    sh = d_nsq
    nc.scalar.activation(sh[:], cd16[:], AF.Ln, bias=EPS)
    nc.scalar.activation(sh[:], sh[:], AF.Exp, scale=gamma2)
    psumt = small.tile([P, 1], F32, tag="psumt")
    nc.vector.tensor_reduce(psumt[:], sh[:], axis=AX.X, op=ALU.add)
    pp_ps_t = spsum.tile([P, 64], F32, tag="sm", name="pp_ps_t")
    pp_ps = pp_ps_t[0:BL, 0:1]
    nc.tensor.matmul(pp_ps[:], d64t[:], psumt[:], start=True, stop=True)
    ppe = small.tile([BL, 1], F32, tag="ppe")
    nc.vector.tensor_scalar(ppe[:], pp_ps[:], EPS, None, op0=ALU.add)
    prc64 = small.tile([BL, 1], F32, tag="prc64")
    nc.vector.reciprocal(prc64[:], ppe[:])
    pr_ps_t = spsum.tile([P, 64], F32, tag="sm", name="pr_ps_t")
    pr_ps = pr_ps_t[:, 0:1]
    nc.tensor.matmul(pr_ps[:], d64_f[:], prc64[:], start=True, stop=True)
    prc2 = small.tile([P, 1], F32, tag="prc2")
    nc.vector.tensor_copy(prc2[:], pr_ps[:])

    nc.scalar.activation(sh[:], sh[:], AF.Copy, scale=prc2[:])
    nc.sync.dma_start(out[:], sh[:])


def build(split_waits=True):
    nc = bass.Bass()
    mem = nc.dram_tensor("memory", [BL, M, D], F32, kind="ExternalInput")
    cs = nc.dram_tensor("controller_state", [BL, C], F32, kind="ExternalInput")
    prev2 = nc.dram_tensor("prev2", [P, MH2], F32, kind="ExternalInput")
    wk_pack = nc.dram_tensor("wk_pack", [P, 2 * D], F32, kind="ExternalInput")
    wh_pack = nc.dram_tensor("wh_pack", [P, 12], F32, kind="ExternalInput")
    b_pack = nc.dram_tensor("b_pack", [1, 6], F32, kind="ExternalInput")
    out = nc.dram_tensor("out", [P, MH2], F32, kind="ExternalOutput")

    eps_t = nc.alloc_sbuf_tensor("const-f32-eps", [128, 1], F32)
    nc.gpsimd.memset(eps_t.ap(), EPS)
    nc.const_aps.aps[(F32, EPS)] = eps_t.ap()
    nc.all_engine_barrier()

    with tile.TileContext(nc) as tc:
        with ExitStack() as ctx:
            tc._body_ctx = ctx
            _body(tc, nc, mem, cs, prev2, wk_pack, b_pack, wh_pack, out)
    if split_waits:
        _split_multiwait(nc)
    return nc


def _split_multiwait(nc, max_waits=1):
    """Walrus ISA structs encode a limited number of semaphore waits per
    instruction. Move all but one wait of any multi-wait instruction onto
    same-engine InstNoOp instructions inserted directly before it."""
    for fn in nc.m.functions:
        for blk in fn.blocks:
            insts = blk.instructions
            idx = 0
            while idx < len(insts):
                inst = insts[idx]
                si = inst.sync_info
                if si is not None and len(si.on_wait) > max_waits:
                    waits = list(si.on_wait)
                    extra, keep = waits[:-max_waits], waits[-max_waits:]
                    for w in extra:
                        nop = mybir.InstNoOp(
                            name=nc.get_next_instruction_name(),
                            sync_info=mybir.SyncInfo(on_wait=[w], on_update=[]),
                            bass_nofuse=True,
                            engine=inst.engine,
                        )
                        insts.insert(idx, nop)
                        idx += 1
                    inst.sync_info = mybir.SyncInfo(
                        on_wait=keep, on_update=list(si.on_update))
                idx += 1


_NC = None


def _get_nc():
    global _NC
    if _NC is None:
        _NC = build()
    return _NC


def _j_of_c():
    # j = 1024*ph + 8*p + v for c = 1024*ph + 128*v + p
    c = np.arange(MH2)
    ph = c // 1024
    v = (c % 1024) // P
    p = c % P
    return 1024 * ph + 8 * p + v


_JC = _j_of_c()


def _make_in_maps(inputs):
    full = {k: np.ascontiguousarray(np.asarray(v, dtype=np.float32))
            for k, v in inputs.items()}
    wk_pack = np.ascontiguousarray(
        np.concatenate([full["Wk"][0:P, :], full["Wk"][P:C, :]], axis=1))
    wh = np.concatenate(
        [full["Wb"], full["Wgate"], full["Ws"], full["Wg"]], axis=1)
    wh_pack = np.ascontiguousarray(np.concatenate([wh[0:P], wh[P:C]], axis=1))
    b_pack = np.ascontiguousarray(np.concatenate(
        [full["bb"].reshape(-1), full["bgate"].reshape(-1),
         full["bs"].reshape(-1), full["bg"].reshape(-1)]).reshape(1, 6))
    in_maps = []
    for c in range(NCORES):
        sl = slice(c * BL, (c + 1) * BL)
        # prev2 rows 2b+s = prev[b, 2*j(c)+s]
        pv = full["previous_weights"][sl]            # [64, 4096]
        pj = pv.reshape(BL, MH2, 2)[:, _JC, :]       # [64, 2048(c), 2(s)]
        prev2 = np.ascontiguousarray(
            pj.transpose(0, 2, 1).reshape(P, MH2))
        in_maps.append({
            "memory": full["memory"][sl],
            "controller_state": full["controller_state"][sl],
            "prev2": prev2,
            "wk_pack": wk_pack, "wh_pack": wh_pack, "b_pack": b_pack,
        })
    return in_maps


def run(inputs, **kwargs):
    from concourse.bass_utils import run_bass_kernel_spmd
    nc = _get_nc()
    res = run_bass_kernel_spmd(nc, _make_in_maps(inputs),
                               list(range(NCORES)), **kwargs)
    inv = np.argsort(_JC)
    outs = []
    for c in range(NCORES):
        o2 = res.results[c]["out"]                   # [128, 2048] rows 2b+s
        o = o2.reshape(BL, 2, MH2)[:, :, inv]        # cols back to j-order
        o = o.transpose(0, 2, 1).reshape(BL, M)      # m = 2j+s
        outs.append(o)
    out = np.concatenate(outs, axis=0)
    return out.astype(np.float32), res


def kernel(**inputs):
    out, _ = run(inputs)
    return out
